# revision 6
# baseline (speedup 1.0000x reference)
"""BiologicalGPT forward pass on 8 Trainium2 NeuronCores.

Sharding: batch (2) -> two groups of 4 cores; within a group tokens are
split 4 x 256. Per layer each core computes qkv for its token chunk,
AllGathers K/V (fp16) across its group, runs causal attention for its
queries over all keys, then the token-local out-proj/FF. The lm_head is
token-parallel (full lm_w on every core). Residual stream is kept
feature-major ([128, 6, 256] fp32) so every projection streams through
the PE without transposes; matmul inputs are fp16 with fp32 PSUM
accumulation. The motif conv bias of odd layers is constant across keys
for a given query, so it cancels in softmax and is dropped. Causal and
dna_bias masking is applied as host-precomputed exp(bias)*tril fp16
multiplier tiles on the transposed scores.
"""

import numpy as np

import concourse.bacc as bacc
import concourse.mybir as mybir
import concourse.tile as tile
from concourse import bass_utils

F16 = mybir.dt.float16
F32 = mybir.dt.float32

VOCAB, DIM, HEADS, HD = 32000, 768, 12, 64
B, T = 2, 1024
NC = 8  # cores
GROUP = 4  # cores per batch group
NT = T // GROUP  # tokens per core = 256
KO = DIM // 128  # 6 feature chunks
NE = 3  # even (dna) layers
NO = 3  # odd layers
L = 6
FF = 4 * DIM  # 3072
KT = T // 128  # 8 key tiles
NVC = VOCAB // 512  # 62.5 -> handle tail
EPS = 1e-5

_BUILD_CACHE = {}


def _build(L_layers=L, n_cores=NC):
    nc = bacc.Bacc("TRN2", target_bir_lowering=False, debug=False, num_devices=n_cores)

    # ---- DRAM I/O ----
    x0_d = nc.dram_tensor("x0", [KO, 128, NT], F32, kind="ExternalInput")
    mE_d = nc.dram_tensor("maskE", [NE, KT, 128, NT], F16, kind="ExternalInput")
    mO_d = nc.dram_tensor("maskO", [KT, 128, NT], F16, kind="ExternalInput")
    wqkv_d = nc.dram_tensor("wqkv", [L_layers, KO, 128, 3 * DIM], F16, kind="ExternalInput")
    wout_d = nc.dram_tensor("wout", [L_layers, KO, 128, DIM], F16, kind="ExternalInput")
    wff1_d = nc.dram_tensor("wff1", [L_layers, KO, 128, FF], F16, kind="ExternalInput")
    wff2_d = nc.dram_tensor("wff2", [L_layers, FF // 128, 128, DIM], F16, kind="ExternalInput")
    wlm_d = nc.dram_tensor("wlm", [KO, 128, VOCAB], F16, kind="ExternalInput")
    y_d = nc.dram_tensor("y", [NT, VOCAB], F32, kind="ExternalOutput")

    rg = [list(range(g * GROUP, (g + 1) * GROUP)) for g in range(n_cores // GROUP)]

    from contextlib import ExitStack

    with tile.TileContext(nc) as tc, ExitStack() as es:
        def _pool(name, bufs, space="SBUF"):
            return es.enter_context(tc.tile_pool(name=name, bufs=bufs, space=space))

        constp = _pool("const", 1)
        xresp = _pool("xres", 1)
        wqkvp = _pool("wqkvp", 1)
        woutp = _pool("woutp", 1)
        wff1p = _pool("wff1p", 1)
        wff2p = _pool("wff2p", 1)
        wlmp = _pool("wlmp", 2)
        actp = _pool("actp", 1)
        kvgp = _pool("kvg", 1)
        maskp = _pool("masks", 2)
        sxp = _pool("sx", 2)
        rowp = _pool("rows", 1)
        expp = _pool("exps", 4)
        evp = _pool("evict", 3)
        psp = _pool("psum", 4, "PSUM")
        psaccp = _pool("psacc", 2, "PSUM")
        psrowp = _pool("psrow", 2, "PSUM")
        dramp = _pool("dram", 2, "DRAM")
        if True:
            # constants
            ones_c16 = constp.tile([128, 1], F16)
            nc.vector.memset(ones_c16[:], 1.0)
            ones_r32 = constp.tile([1, 128], F32)
            nc.vector.memset(ones_r32[:], 1.0)
            m0r = constp.tile([1, 128], F16)
            nc.vector.memset(m0r[:], 0.0)
            nc.vector.memset(m0r[:, 0:64], 1.0)
            m1r = constp.tile([1, 128], F16)
            nc.vector.memset(m1r[:], 0.0)
            nc.vector.memset(m1r[:, 64:128], 1.0)
            eps_c = constp.tile([1, 1], F32)
            nc.vector.memset(eps_c[:], EPS)
            zero_p = constp.tile([128, 1], F32)
            nc.vector.memset(zero_p[:], 0.0)

            # persistent tensors
            x = xresp.tile([128, KO, NT], F32)
            nc.sync.dma_start(x[:], x0_d.ap().rearrange("ko p t -> p ko t"))

            def layer_norm(x_ap, n_ko):
                """Returns z [128, n_ko, NT] fp16 = normalized x (gamma/beta
                folded into weights host-side)."""
                z = sxp.tile([128, n_ko, NT], F16, tag="z")
                stats = psrowp.tile([1, 512], F32, tag="rowps")
                for ko in range(n_ko):
                    sx = sxp.tile([128, 2 * NT], F16, tag="sx")
                    nc.vector.tensor_copy(sx[:, 0:NT], x_ap[:, ko, :])
                    nc.vector.tensor_mul(
                        sx[:, NT : 2 * NT], x_ap[:, ko, :], x_ap[:, ko, :]
                    )
                    nc.tensor.matmul(
                        stats[:], ones_c16[:], sx[:],
                        start=(ko == 0), stop=(ko == n_ko - 1),
                    )
                # rows: mu, msq, var, std, rstd, mu*rstd
                rowbuf = rowp.tile([1, 2 * NT], F32, tag="rowbuf")  # [rstd | mu*rstd]
                mu = rowp.tile([1, NT], F32, tag="mu")
                var = rowp.tile([1, NT], F32, tag="var")
                nc.scalar.activation(
                    mu[:], stats[:, 0:NT],
                    mybir.ActivationFunctionType.Copy, scale=1.0 / DIM,
                )
                nc.scalar.activation(
                    var[:], stats[:, NT : 2 * NT],
                    mybir.ActivationFunctionType.Copy, scale=1.0 / DIM,
                )
                musq = rowp.tile([1, NT], F32, tag="musq")
                nc.vector.tensor_mul(musq[:], mu[:], mu[:])
                nc.vector.tensor_sub(var[:], var[:], musq[:])
                std = rowp.tile([1, NT], F32, tag="std")
                nc.scalar.activation(
                    std[:], var[:], mybir.ActivationFunctionType.Sqrt, bias=eps_c[:]
                )
                nc.vector.reciprocal(rowbuf[:, 0:NT], std[:])
                nc.vector.tensor_mul(rowbuf[:, NT : 2 * NT], mu[:], rowbuf[:, 0:NT])
                bc = psrowp.tile([128, 2 * NT], F32, tag="rowps")
                nc.tensor.matmul(bc[:], ones_r32[:], rowbuf[:], start=True, stop=True)
                tmp = sxp.tile([128, NT], F32, tag="lntmp")
                for ko in range(n_ko):
                    nc.vector.tensor_mul(tmp[:], x_ap[:, ko, :], bc[:, 0:NT])
                    nc.vector.tensor_sub(z[:, ko, :], tmp[:], bc[:, NT : 2 * NT])
                return z

            for li in range(L_layers):
                even = li % 2 == 0
                le = li // 2

                # ---- LN1 + qkv ----
                z = layer_norm(x, KO)
                wqkv = wqkvp.tile([128, KO, 2 * DIM], F16, tag="wqkv")
                nc.sync.dma_start(
                    wqkv[:], wqkv_d[li, :, :, 0 : 2 * DIM].rearrange("ko p f -> p ko f")
                )
                mk = maskp.tile([128, KT, NT], F16, tag="mk")
                if even:
                    nc.sync.dma_start(mk[:], mE_d[le].rearrange("kt p t -> p kt t"))
                else:
                    nc.sync.dma_start(mk[:], mO_d.ap().rearrange("kt p t -> p kt t"))

                qk = actp.tile([128, 2 * KO, NT], F16, tag="qk")  # q chunks 0-5, k 6-11
                for oc in range(2 * KO):
                    ps = psp.tile([128, NT], F32, tag="ps")
                    for ko in range(KO):
                        nc.tensor.matmul(
                            ps[:], wqkv[:, ko, oc * 128 : (oc + 1) * 128], z[:, ko, :],
                            start=(ko == 0), stop=(ko == KO - 1),
                        )
                    nc.scalar.copy(qk[:, oc, :], ps[:])
                wqkv_v = wqkvp.tile([128, KO, DIM], F16, tag="wqkv")
                nc.sync.dma_start(
                    wqkv_v[:],
                    wqkv_d[li, :, :, 2 * DIM : 3 * DIM].rearrange("ko p f -> p ko f"),
                )
                v = actp.tile([128, 2, DIM], F16, tag="v")  # token-major
                for mt in range(2):
                    for nf in range(2):
                        nn = 512 if nf == 0 else 256
                        ps = psp.tile([128, 512], F32, tag="ps")
                        for ko in range(KO):
                            nc.tensor.matmul(
                                ps[:, 0:nn],
                                z[:, ko, mt * 128 : (mt + 1) * 128],
                                wqkv_v[:, ko, nf * 512 : nf * 512 + nn],
                                start=(ko == 0), stop=(ko == KO - 1),
                            )
                        nc.scalar.copy(v[:, mt, nf * 512 : nf * 512 + nn], ps[:, 0:nn])

                # ---- AllGather K,V (fp16) over the 4-core group ----
                kv_in = dramp.tile([2 * DIM, NT], F16, tag="kvin")
                nc.sync.dma_start(
                    kv_in[0:DIM, :].rearrange("(ko p) t -> p ko t", p=128),
                    qk[:, KO : 2 * KO, :],
                )
                nc.sync.dma_start(
                    kv_in[DIM : 2 * DIM, :].rearrange("(mt p) f -> p mt f", p=128),
                    v[:],
                )
                kv_out = dramp.tile([GROUP, 2 * DIM, NT], F16, tag="kvout")
                nc.gpsimd.collective_compute(
                    "AllGather",
                    mybir.AluOpType.bypass,
                    replica_groups=rg,
                    ins=[kv_in.opt()],
                    outs=[kv_out.opt()],
                )
                kg = kvgp.tile([128, KO, T], F16, tag="kg")
                vg = kvgp.tile([128, KT, DIM], F16, tag="vg")
                for r in range(GROUP):
                    nc.sync.dma_start(
                        kg[:, :, r * NT : (r + 1) * NT],
                        kv_out[r, 0:DIM, :].rearrange("(ko p) t -> p ko t", p=128),
                    )
                    nc.sync.dma_start(
                        vg[:, 2 * r : 2 * r + 2, :],
                        kv_out[r, DIM : 2 * DIM, :].rearrange(
                            "(mt p) f -> p mt f", p=128
                        ),
                    )

                # ---- attention ----
                att = actp.tile([128, KO, NT], F16, tag="att")
                rc = rowp.tile([1, 2, NT], F16, tag="rc")
                for h in range(HEADS):
                    ko, hf = h // 2, (h % 2) * 64
                    if h % 2 == 0:
                        av = psaccp.tile([128, NT], F32, tag="av")
                    dn = psrowp.tile([1, NT], F32, tag="rowps")
                    for kt in range(KT):
                        sc = psp.tile([128, NT], F32, tag="ps")
                        nc.tensor.matmul(
                            sc[:],
                            kg[hf : hf + 64, ko, kt * 128 : (kt + 1) * 128],
                            qk[hf : hf + 64, ko, :],
                            start=True, stop=True,
                        )
                        et = expp.tile([128, NT], F16, tag="et")
                        nc.scalar.activation(
                            et[:], sc[:], mybir.ActivationFunctionType.Exp, bias=zero_p[:], scale=0.125
                        )
                        mask_ap = mk[:, kt, :]
                        nc.vector.tensor_mul(et[:], et[:], mask_ap)
                        nc.tensor.matmul(
                            dn[:], ones_c16[:], et[:],
                            start=(kt == 0), stop=(kt == KT - 1),
                        )
                        nc.tensor.matmul(
                            av[hf : hf + 64, :],
                            vg[:, kt, h * 64 : (h + 1) * 64],
                            et[:],
                            start=(kt == 0), stop=(kt == KT - 1),
                            tile_position=(0, hf),
                        )
                    with nc.allow_low_precision(reason="fp16 softmax recip"):
                        nc.vector.reciprocal(rc[:, h % 2, :], dn[:])
                    if h % 2 == 1:
                        rb = psp.tile([128, NT], F32, tag="ps")
                        nc.tensor.matmul(rb[:], m0r[:], rc[:, 0, :], start=True, stop=False)
                        nc.tensor.matmul(rb[:], m1r[:], rc[:, 1, :], start=False, stop=True)
                        rbs = evp.tile([128, NT], F16, tag="rbs")
                        nc.scalar.copy(rbs[:], rb[:])
                        nc.vector.tensor_mul(att[:, ko, :], av[:], rbs[:])

                # ---- out proj + residual ----
                wout = woutp.tile([128, KO, DIM], F16)
                nc.sync.dma_start(wout[:], wout_d[li].rearrange("ko p f -> p ko f"))
                for oc in range(KO):
                    ps = psp.tile([128, NT], F32, tag="ps")
                    for ko in range(KO):
                        nc.tensor.matmul(
                            ps[:], wout[:, ko, oc * 128 : (oc + 1) * 128], att[:, ko, :],
                            start=(ko == 0), stop=(ko == KO - 1),
                        )
                    nc.vector.tensor_add(x[:, oc, :], x[:, oc, :], ps[:])

                # ---- LN2 + FF ----
                z2 = layer_norm(x, KO)
                hidden = actp.tile([128, FF // 128, NT], F16, tag="hidden")
                for hf in range(2):
                    wff1 = wff1p.tile([128, KO, FF // 2], F16, tag="wff1")
                    nc.sync.dma_start(
                        wff1[:],
                        wff1_d[li, :, :, hf * (FF // 2) : (hf + 1) * (FF // 2)].rearrange(
                            "ko p f -> p ko f"
                        ),
                    )
                    for oc0 in range(12):
                        oc = hf * 12 + oc0
                        ps = psp.tile([128, NT], F32, tag="ps")
                        for ko in range(KO):
                            nc.tensor.matmul(
                                ps[:], wff1[:, ko, oc0 * 128 : (oc0 + 1) * 128], z2[:, ko, :],
                                start=(ko == 0), stop=(ko == KO - 1),
                            )
                        nc.scalar.activation(
                            hidden[:, oc, :], ps[:],
                            mybir.ActivationFunctionType.Gelu, bias=zero_p[:],
                        )
                for hf in range(2):
                    wff2 = wff2p.tile([128, 12, DIM], F16, tag="wff2")
                    nc.sync.dma_start(
                        wff2[:],
                        wff2_d[li, hf * 12 : (hf + 1) * 12].rearrange("ko p f -> p ko f"),
                    )
                    for oc in range(KO):
                        ps = psp.tile([128, NT], F32, tag="ps")
                        for kt0 in range(12):
                            nc.tensor.matmul(
                                ps[:], wff2[:, kt0, oc * 128 : (oc + 1) * 128],
                                hidden[:, hf * 12 + kt0, :],
                                start=(kt0 == 0), stop=(kt0 == 11),
                            )
                        nc.vector.tensor_add(x[:, oc, :], x[:, oc, :], ps[:])

            # ---- final LN + lm_head ----
            zf = layer_norm(x, KO)
            n_chunks = (VOCAB + 511) // 512
            for vc in range(n_chunks):
                nn = min(512, VOCAB - vc * 512)
                wlm = wlmp.tile([128, KO, 512], F16, tag="wlm")
                for ko in range(KO):
                    nc.sync.dma_start(
                        wlm[:, ko, 0:nn], wlm_d[ko, :, vc * 512 : vc * 512 + nn]
                    )
                for mt in range(2):
                    ps = psp.tile([128, 512], F32, tag="ps")
                    for ko in range(KO):
                        nc.tensor.matmul(
                            ps[:, 0:nn],
                            zf[:, ko, mt * 128 : (mt + 1) * 128],
                            wlm[:, ko, 0:nn],
                            start=(ko == 0), stop=(ko == KO - 1),
                        )
                    ev = evp.tile([128, 512], F32, tag="lmev")
                    nc.vector.tensor_copy(ev[:, 0:nn], ps[:, 0:nn])
                    nc.sync.dma_start(
                        y_d[mt * 128 : (mt + 1) * 128, vc * 512 : vc * 512 + nn],
                        ev[:, 0:nn],
                    )

    nc.compile()
    return nc


def _get_nc(L_layers=L, n_cores=NC):
    key = (L_layers, n_cores)
    if key not in _BUILD_CACHE:
        _BUILD_CACHE[key] = _build(L_layers, n_cores)
    return _BUILD_CACHE[key]


def _host_prepare(tokens, params, L_layers=L):
    p = params
    tokens = np.asarray(tokens).astype(np.int64)
    te = np.asarray(p["token_emb"], np.float32)
    x0 = te[tokens] + np.asarray(p["pos_emb"], np.float32)[:T][None]
    x0 = x0 + np.asarray(p["helical"], np.float32)[:T][None]  # [B, T, DIM]

    blocks = p["blocks"]

    def fold(g, b, w, bias):
        w = np.asarray(w, np.float32)
        g = np.asarray(g, np.float32)
        b = np.asarray(b, np.float32)
        bias = np.asarray(bias, np.float32)
        wf = w * g[:, None]
        bf = bias + b @ w
        return wf.astype(np.float16), bf

    wqkv = np.zeros((L_layers, KO, 128, 3 * DIM), np.float16)
    wout = np.zeros((L_layers, KO, 128, DIM), np.float16)
    wff1 = np.zeros((L_layers, KO, 128, FF), np.float16)
    wff2 = np.zeros((L_layers, FF // 128, 128, DIM), np.float16)
    max_bias = 0.0
    for li in range(L_layers):
        blk = blocks[li]
        wq, bq = fold(blk["ln1_g"], blk["ln1_b"], blk["qkv_w"], blk["qkv_b"])
        wqkv[li] = wq.reshape(KO, 128, 3 * DIM)
        wo = np.asarray(blk["out_w"], np.float32).astype(np.float16)
        wout[li] = wo.reshape(KO, 128, DIM)
        w1, b1 = fold(blk["ln2_g"], blk["ln2_b"], blk["ff1_w"], blk["ff1_b"])
        wff1[li] = w1.reshape(KO, 128, FF)
        w2 = np.asarray(blk["ff2_w"], np.float32).astype(np.float16)
        wff2[li] = w2.reshape(FF // 128, 128, DIM)
        max_bias = max(
            max_bias,
            np.abs(bq).max(),
            np.abs(np.asarray(blk["out_b"])).max(),
            np.abs(b1).max(),
            np.abs(np.asarray(blk["ff2_b"])).max(),
        )
    wlm, blm = fold(p["ln_f_g"], p["ln_f_b"], p["lm_w"], p["lm_b"])
    wlm = wlm.reshape(KO, 128, VOCAB)
    if max_bias > 0 or np.abs(blm).max() > 0:
        raise NotImplementedError("nonzero projection biases not supported")

    # masks, per core: expb[k, q] = exp(dna_bias[q_global, k_global]) * (k<=q)
    per_core = []
    for c in range(NC):
        b_, j = c // GROUP, c % GROUP
        q0 = j * NT
        qg = np.arange(q0, q0 + NT)
        kgl = np.arange(T)
        tri = (kgl[:, None] <= qg[None, :]).astype(np.float32)  # [T, NT]
        mE_ = np.zeros((NE, KT, 128, NT), np.float16)
        for le in range(NE):
            if 2 * le >= L_layers:
                continue
            dna = np.asarray(blocks[2 * le]["dna_bias"], np.float32)[:T, :T]
            m = np.exp(dna[qg][:, :].T) * tri  # [T, NT]
            mE_[le] = m.reshape(KT, 128, NT).astype(np.float16)
        mO_ = tri.reshape(KT, 128, NT).astype(np.float16)
        x0c = np.ascontiguousarray(
            x0[b_, q0 : q0 + NT].T.reshape(KO, 128, NT), dtype=np.float32
        )
        per_core.append(
            {
                "x0": x0c,
                "maskE": mE_,
                "maskO": mO_,
                "wqkv": wqkv,
                "wout": wout,
                "wff1": wff1,
                "wff2": wff2,
                "wlm": wlm,
            }
        )
    return per_core


def kernel(tokens, params, _trace=False, _L=L):
    nc = _get_nc(_L, NC)
    ins = _host_prepare(tokens, params, _L)
    res = bass_utils.run_bass_kernel_spmd(
        nc, ins, core_ids=list(range(NC)), trace=_trace
    )
    out = np.empty((B, T, VOCAB), np.float32)
    for c in range(NC):
        b_, j = c // GROUP, c % GROUP
        out[b_, j * NT : (j + 1) * NT] = res.results[c]["y"]
    if _trace:
        kernel.last_exec_time_ns = res.exec_time_ns
        kernel.last_trace = res.instructions_and_trace
    return out
